# revision 34
# baseline (speedup 1.0000x reference)
"""Trainium2 Bass kernel for the BatteryCell elementwise model.

Pure data-parallel over the batch: the 2,097,152-element batch is split
across 8 NeuronCores (262,144 each). Each core sees its shard as
[128 partitions x 2048 free] and streams it in tiles of FD columns.

Layout: `states` [Bc, 8] is DMA'd interleaved (contiguous per partition)
and the 8 state planes are accessed in place with stride-8 access
patterns; the two electrodes (n at e=5, p at e=7 etc.) are processed as
[128, FD, 2] "pair" APs so each ACT/DVE instruction covers both.

All transcendentals use only Ln / Exp / Square / Copy so the scalar
engine stays on one activation table the whole kernel (no table reloads):
  u^-1/2      = exp(-0.5*(ln q + ln(1 - q/qSMax)))   (no reciprocal op)
  arcsinh(z)  = ln(z + exp(0.5*ln(1 + z^2)))
  ln((1-x)/x) = ln(1-x) - ln(x)
The small MLPp (1->8->4->1 tanh net, a scalar function of xpS2 once its
weights are fixed) is evaluated as a build-time-fitted polynomial in
qpS_new with fused scalar_tensor_tensor Horner steps on the vector
engine. Inputs that are constant arrays in practice (qMax, Ro) and the
scalar tDiffusion plus the MLP weights are folded into immediates at
build time (the build is re-specialized per call from the actual numpy
values, with a general per-element fallback path when they are not
constant).
"""

import math

import numpy as np

import concourse.bass as bass
import concourse.mybir as mybir
from concourse import tile
from concourse.bass_utils import run_bass_kernel_spmd

# ---------------------------------------------------------------- constants
R_GAS = 8.3144621
F = 96487.0
ALPHA = 0.5
VOL = 2.2e-5
VOLS = 0.1 * VOL
VOLB = VOL - VOLS
SN = 2e-4
SP = 2e-4
KN = 2e4
KP = 2e4
TO = 10.0
TSN = 90.0
TSP = 90.0
U0P = 4.03
U0N = 0.01
QMAX_BASE = 10000.0
RO_BASE = 10.0
DT = 1.0

B = 2097152
NCORES = 8
BC = B // NCORES          # 262144 per core
P = 128
NTOT = BC // P            # 2048 free elements per partition
FD = 512                  # free elements per tile
NT = NTOT // FD           # tiles per core

f32 = mybir.dt.float32
AF = mybir.ActivationFunctionType
OP = mybir.AluOpType

# engine assignment for the TT-class ops that can move between engines.
# 'V' = vector (DVE), 'G' = gpsimd. Tuned from profiling.
ENG = {
    "qB_new": "G",
    "tb_copy": "G",
    "dd": "G",
    "d3": "V",
    "u1": "G",
    "u2": "G",
    "v2": "G",
    "hn": "V",
    "hp": "V",
    "vfin": "G",
    "qsn": "V",
    "lu": "G",
    "zz": "V",
    "ss": "V",
}

# engine map for the final tile's epilogue: everything on the fast
# engines (DVE), since they are idle during the tail
ENG_TAIL = {k: ("V" if v == "G" else v) for k, v in ENG.items()}
ENG_TAIL["tb_copy"] = "G"
ENG_TAIL["lu"] = "G"

# head tile uses the steady-state map (measured best)
ENG_HEAD = dict(ENG)

# head/tail-tapered tile sizes (sum = NTOT) chosen via cost-model sweeps
TILE_SIZES = [128, 256, 512, 512, 512, 128]

_EVSEM_CAP = 2
_NO_SLOT = {"InstDrain", "InstNoOp"}


def _legalize_waits(nc, default_cap=1, evsem_cap=_EVSEM_CAP):
    """walrus cannot encode >cap sem waits on one instruction (and none at
    all on Drain/NoOp). Hoist excess waits onto InstEventSemaphore
    instructions inserted before the offender on the same engine."""
    nid = [0]
    for fn in nc.m.functions:
        for bb in fn.blocks:
            insts = bb.instructions
            i = 0
            while i < len(insts):
                ins = insts[i]
                si = ins.sync_info
                waits = list(si.on_wait) if si and si.on_wait else []
                cls = ins.__class__.__name__
                if cls in _NO_SLOT:
                    cap = 0
                elif cls == "InstEventSemaphore":
                    cap = evsem_cap
                else:
                    cap = default_cap
                if len(waits) > cap:
                    keep = waits[len(waits) - cap:] if cap else []
                    extra = waits[:len(waits) - cap] if cap else waits
                    new = []
                    for j in range(0, len(extra), evsem_cap):
                        ev = mybir.InstEventSemaphore(
                            name=f"WLG-{nid[0]}", ins=[], outs=[])
                        nid[0] += 1
                        ev.engine = ins.engine
                        ev.sync_info = mybir.SyncInfo(
                            on_wait=extra[j:j + evsem_cap], on_update=[])
                        new.append(ev)
                    ins.sync_info = mybir.SyncInfo(
                        on_wait=keep,
                        on_update=list(si.on_update) if si.on_update else [])
                    for k, ev in enumerate(new):
                        insts.insert(i + k, ev)
                    i += len(new)
                i += 1


# ------------------------------------------------------------- MLP poly fit
def _mlp_f(x, Wp1, bp1, Wp2, bp2, Wp3, bp3):
    """The exact MLPp as a scalar function of x (float64 numpy)."""
    h = np.tanh(np.outer(x, Wp1[0]) + bp1)
    h = np.tanh(h @ Wp2 + bp2)
    return (h @ Wp3 + bp3)[:, 0]


def _fit_mlp_poly(qlo, qhi, rqs, Wp1, bp1, Wp2, bp2, Wp3, bp3,
                  tol=2e-7, max_deg=14):
    """Fit F(q) = MLPp(rqs*q) over q in [qlo, qhi] as a polynomial in
    t = (q - mid)/hw. Returns (mid, hw, coeffs c_0..c_D)."""
    mid = 0.5 * (qlo + qhi)
    hw = 0.5 * (qhi - qlo)
    t = np.cos(np.pi * (np.arange(2000) + 0.5) / 2000)  # cheb nodes in [-1,1]
    q = mid + hw * t
    y = _mlp_f(q * rqs, Wp1, bp1, Wp2, bp2, Wp3, bp3)
    for deg in range(3, max_deg + 1):
        c = np.polynomial.chebyshev.chebfit(t, y, deg)
        err = np.abs(np.polynomial.chebyshev.chebval(t, c) - y).max()
        if err < tol:
            break
    pow_c = np.polynomial.chebyshev.cheb2poly(c)  # c_0..c_D in power basis
    return mid, hw, pow_c, err


# ---------------------------------------------------------------- builder
def build_kernel(consts, fd=FD, ntot=NTOT, general_q=False, general_ro=False,
                 im_bufs=1, st_bufs=3, it_bufs=3, xo_bufs=2, vo_bufs=2,
                 tile_sizes=None, b_lag=1):
    """Build the Bass module for one core's shard ([128, ntot] per plane).

    consts: dict with rqs (1/qSMax) or None, kro (Ro*RO_BASE*DT/TO) or
    None, c1, c2 (charge-diffusion constants), poly (mid, hw, coeffs),
    wn_eff (Wn*rqs), k0 (constant fold).
    """
    nt = ntot // fd
    nc = bass.Bass(trn_type="TRN2")

    st_d = nc.dram_tensor("states", [P * ntot, 8], f32, kind="ExternalInput")
    it_d = nc.dram_tensor("inputs", [P * ntot, 1], f32, kind="ExternalInput")
    if general_q:
        qm_d = nc.dram_tensor("qMax", [P * ntot], f32, kind="ExternalInput")
    if general_ro:
        ro_d = nc.dram_tensor("Ro", [P * ntot], f32, kind="ExternalInput")
    xo_d = nc.dram_tensor("Xnew", [P * ntot, 8], f32, kind="ExternalOutput")
    vo_d = nc.dram_tensor("V", [P * ntot, 1], f32, kind="ExternalOutput")

    st_v = st_d.rearrange("(p n) e -> p n e", p=P)
    it_v = it_d.rearrange("(p n) e -> p (n e)", p=P)
    xo_v = xo_d.rearrange("(p n) e -> p n e", p=P)
    vo_v = vo_d.rearrange("(p n) e -> p (n e)", p=P)
    if general_q:
        qm_v = qm_d.rearrange("(p n) -> p n", p=P)
    if general_ro:
        ro_v = ro_d.rearrange("(p n) -> p n", p=P)

    rqs = consts.get("rqs")
    kro = consts.get("kro")
    c1, c2 = consts["c1"], consts["c2"]
    mid, hw, pc = consts["poly"]
    wn_eff = consts["wn_eff"]
    k0 = consts["k0"]
    C1 = R_GAS / (F * ALPHA)
    LN8TH_N = math.log(1.0 / (2.0 * SN * KN))
    LN8TH_P = math.log(1.0 / (2.0 * SP * KP))
    assert abs(LN8TH_N - LN8TH_P) < 1e-12  # same fold for the n/p pair

    deg = len(pc) - 1
    # reversed-Horner params: G1 = A1*t + A0; G_{j+1} = (G_j + A_{j+1})*t
    # covers all degrees >= 1; constant pc[0] is folded into k0 upstream.
    A = [0.0] * (deg + 1)
    A[1] = float(pc[deg])
    A[0] = float(pc[deg - 1])
    for j in range(2, deg):
        A[j + 1] = float(pc[deg - j])
    # (A[2] stays 0; last A consumed at j = deg-1 -> A[deg] = pc[1])

    with tile.TileContext(nc) as tc:
        with (
            tc.tile_pool(name="io_st", bufs=st_bufs) as p_st,
            tc.tile_pool(name="io_it", bufs=it_bufs) as p_it,
            tc.tile_pool(name="io_xo", bufs=xo_bufs) as p_xo,
            tc.tile_pool(name="io_vo", bufs=vo_bufs) as p_vo,
            tc.tile_pool(name="im", bufs=im_bufs) as im,
            tc.tile_pool(name="imq", bufs=max(2, im_bufs)) as imq,
            tc.tile_pool(name="cst", bufs=1) as cst,
        ):
            ln8 = cst.tile([P, 1], f32, tag="ln8")
            nc.gpsimd.memset(ln8[:], LN8TH_N)

            eng = {"V": nc.vector, "G": nc.gpsimd}

            def stage_a(t, off, fdt, eng_map=ENG):
                """DMA-in + kinetics + state updates; writes all of Xnew."""
                sl = slice(off, off + fdt)
                st = p_st.tile([P, fdt * 8], f32, tag="st", name="st")
                nc.sync.dma_start(
                    st.rearrange("p (n e) -> p n e", e=8), st_v[:, sl, :])
                it = p_it.tile([P, fdt], f32, tag="it", name="it")
                nc.sync.dma_start(it, it_v[:, sl])
                qm = ro = None
                if general_q:
                    qm = p_it.tile([P, fdt], f32, tag="qm", name="qm")
                    nc.sync.dma_start(qm, qm_v[:, sl])
                if general_ro:
                    ro = p_it.tile([P, fdt], f32, tag="ro", name="ro")
                    nc.sync.dma_start(ro, ro_v[:, sl])
                xo = p_xo.tile([P, fdt * 8], f32, tag="xo", name="xo")

                s3 = st.rearrange("p (n e) -> p n e", e=8)
                x3 = xo.rearrange("p (n e) -> p n e", e=8)
                sS = s3[:, :, 5:8:2]     # (qnS, qpS) pair
                sB = s3[:, :, 4:8:2]     # (qnB, qpB) pair
                itb = it.broadcast_to((P, fdt, 2))

                def pair(tag):
                    tl = im.tile([P, fdt * 2], f32, tag=tag, name=tag)
                    return tl.rearrange("p (n e) -> p n e", e=2)

                def plane(tag):
                    return im.tile([P, fdt], f32, tag=tag, name=tag)

                lnqs_b = qs_b = lnqs = None
                if general_q:
                    lnqs = plane("lnqs")
                    nc.scalar.activation(lnqs, qm, AF.Ln, scale=1000.0)
                    lnqs_b = lnqs.broadcast_to((P, fdt, 2))

                # ---- electrode kinetics on old states (paired n/p) ----
                l1 = pair("l1")
                l2 = pair("l2")
                if general_q:
                    qs = plane("qs")
                    nc.scalar.activation(qs, qm, AF.Copy, scale=1000.0)
                    qs_b = qs.broadcast_to((P, fdt, 2))
                    dq = pair("dq")
                    nc.vector.tensor_sub(dq, qs_b, sS)
                    nc.scalar.activation(l1, sS, AF.Ln)
                    nc.scalar.activation(l2, dq, AF.Ln)
                    lu0 = pair("lu0")
                    nc.vector.tensor_add(lu0, l1, l2)
                    lu = pair("lu")
                    nc.vector.scalar_tensor_tensor(
                        lu, lnqs_b, -2.0, lu0, OP.mult, OP.add)
                else:
                    nc.scalar.activation(l1, sS, AF.Ln, scale=rqs)
                    nc.scalar.activation(l2, sS, AF.Ln, scale=-rqs, bias=1.0)
                    lu = pair("lu")
                    eng[eng_map["lu"]].tensor_add(lu, l1, l2)
                ee = pair("ee")
                nc.scalar.activation(ee, lu, AF.Exp, scale=-0.5, bias=ln8[:])
                zz = pair("zz")
                eng[eng_map["zz"]].tensor_mul(zz, ee, itb)
                sq = pair("sq")
                nc.scalar.activation(sq, zz, AF.Square)
                lz = pair("lz")
                nc.scalar.activation(lz, sq, AF.Ln, bias=1.0)
                ww = pair("ww")
                nc.scalar.activation(ww, lz, AF.Exp, scale=0.5)
                ss = pair("ss")
                eng[eng_map["ss"]].tensor_add(ss, zz, ww)
                aa = pair("aa")                       # asinh pair (n, p)
                nc.scalar.activation(aa, ss, AF.Ln)

                # ---- V-state updates ----
                tbb = s3[:, :, 0].broadcast_to((P, fdt, 2))
                n90 = pair("n90")
                nc.vector.scalar_tensor_tensor(
                    n90, tbb, C1 / TSN, aa, OP.mult, OP.mult)  # VsNom/90
                nc.vector.scalar_tensor_tensor(
                    x3[:, :, 2:4], s3[:, :, 2:4], (TSN - 1.0) / TSN, n90,
                    OP.mult, OP.add)
                ta = plane("ta")
                if general_ro:
                    nc.vector.scalar_tensor_tensor(
                        ta, it, RO_BASE * DT / TO, ro, OP.mult, OP.mult)
                else:
                    nc.scalar.activation(ta, it, AF.Copy, scale=kro)
                nc.vector.scalar_tensor_tensor(
                    x3[:, :, 1], s3[:, :, 1], 1.0 - DT / TO, ta,
                    OP.mult, OP.add)

                # ---- charge updates ----
                # qd2 = (c2/c1)*qS - qB = -qdot*DT/c1  (c2/c1 = VOLB/VOLS)
                qd = pair("qd")
                nc.vector.scalar_tensor_tensor(
                    qd, sS, c2 / c1, sB, OP.mult, OP.subtract)
                nc.vector.scalar_tensor_tensor(
                    x3[:, :, 4:8:2], qd, c1, sB, OP.mult, OP.add)
                hn = plane("hn")
                nc.vector.scalar_tensor_tensor(
                    hn, qd[:, :, 0], -c1, it, OP.mult, OP.subtract)
                hp = plane("hp")
                nc.vector.scalar_tensor_tensor(
                    hp, qd[:, :, 1], -c1, it, OP.mult, OP.add)
                eng[eng_map["qsn"]].tensor_add(x3[:, :, 5], s3[:, :, 5], hn)
                eng[eng_map["qsn"]].tensor_add(x3[:, :, 7], s3[:, :, 7], hp)
                eng[eng_map["tb_copy"]].tensor_copy(x3[:, :, 0], s3[:, :, 0])
                nc.sync.dma_start(
                    xo_v[:, sl, :], xo.rearrange("p (n e) -> p n e", e=8))
                return dict(sl=sl, fdt=fdt, xo=xo, x3=x3,
                            lnqs=lnqs, lnqs_b=lnqs_b, qs_b=qs_b)

            def stage_b(ctx, eng_map=ENG):
                """Output voltage from Xnew + DMA-out."""
                sl, fdt = ctx["sl"], ctx["fdt"]
                x3, xo = ctx["x3"], ctx["xo"]
                vo = p_vo.tile([P, fdt], f32, tag="vo", name="vo")

                def pair(tag):
                    tl = im.tile([P, fdt * 2], f32, tag=tag, name=tag)
                    return tl.rearrange("p (n e) -> p n e", e=2)

                def plane(tag):
                    return im.tile([P, fdt], f32, tag=tag, name=tag)

                xS = x3[:, :, 5:8:2]                    # (qnS_new, qpS_new)
                m1 = pair("m1")
                m2 = pair("m2")
                if general_q:
                    dq2 = pair("dq2")
                    nc.vector.tensor_sub(dq2, ctx["qs_b"], xS)
                    nc.scalar.activation(m1, xS, AF.Ln)
                    nc.scalar.activation(m2, dq2, AF.Ln)
                else:
                    nc.scalar.activation(m1, xS, AF.Ln, scale=rqs)
                    nc.scalar.activation(m2, xS, AF.Ln, scale=-rqs, bias=1.0)
                dd = pair("dd")
                eng[eng_map["dd"]].tensor_sub(dd, m2, m1)   # ln((1-x)/x) pair
                d3 = plane("d3")
                eng[eng_map["d3"]].tensor_sub(d3, dd[:, :, 1], dd[:, :, 0])
                vv = plane("vv")
                nc.vector.scalar_tensor_tensor(
                    vv, x3[:, :, 0], C1 * 0.5, d3, OP.mult, OP.mult)
                vvk = plane("vvk")
                nc.scalar.activation(vvk, vv, AF.Copy, bias=k0)

                # ---- MLP polynomial in qpS_new ----
                xx2 = None
                if general_q:
                    lnx2 = pair("lnx2")
                    nc.scalar.activation(lnx2, xS, AF.Ln)
                    xq2 = pair("xq2")
                    nc.vector.scalar_tensor_tensor(
                        xq2, ctx["lnqs_b"], -1.0, lnx2, OP.mult, OP.add)
                    xx2 = pair("xx2")
                    nc.scalar.activation(xx2, xq2, AF.Exp)  # x2 pair
                    tq = plane("tq")
                    nc.vector.tensor_scalar(
                        tq, xx2[:, :, 1], 1.0 / hw, -mid / hw,
                        OP.mult, OP.add)
                else:
                    tq = plane("tq")
                    nc.scalar.activation(tq, x3[:, :, 7], AF.Copy,
                                         scale=1.0 / hw, bias=-mid / hw)
                g = imq.tile([P, fdt], f32, tag="gpoly", name="gpoly")
                nc.vector.tensor_scalar(g, tq, A[1], A[0], OP.mult, OP.add)
                for j in range(2, deg + 1):
                    g2 = imq.tile([P, fdt], f32, tag=f"gpoly{j % 2}",
                                  name=f"gpoly{j % 2}")
                    nc.vector.scalar_tensor_tensor(
                        g2, g, A[j], tq, OP.add, OP.mult)
                    g = g2

                # ---- assemble V ----
                v1 = plane("v1")
                xn_src = xx2[:, :, 0] if general_q else x3[:, :, 5]
                nc.vector.scalar_tensor_tensor(
                    v1, xn_src, -wn_eff, g, OP.mult, OP.add)
                v2 = plane("v2")
                eng[eng_map["v2"]].tensor_add(v2, v1, vvk)
                u1 = plane("u1")
                eng[eng_map["u1"]].tensor_add(u1, x3[:, :, 1], x3[:, :, 2])
                u2 = plane("u2")
                eng[eng_map["u2"]].tensor_add(u2, u1, x3[:, :, 3])
                eng[eng_map["vfin"]].tensor_sub(vo, v2, u2)

                nc.sync.dma_start(vo_v[:, sl], vo)

            # 2-stage software pipeline over the tiles: stage_b of tile t
            # is emitted between stage_a of t+1 and t+2, so every engine's
            # in-order queue interleaves independent work.
            sizes = tile_sizes if tile_sizes is not None else [fd] * nt
            assert sum(sizes) == ntot
            ctxs = []
            off = 0
            nts = len(sizes)
            for i, fdt in enumerate(sizes):
                am = ENG_HEAD if i == 0 else ENG
                ctxs.append(stage_a(i, off, fdt, eng_map=am))
                off += fdt
                if i >= b_lag:
                    stage_b(ctxs[i - b_lag])
            for j in range(nts - b_lag, nts):
                stage_b(ctxs[j],
                        eng_map=ENG_TAIL if j == nts - 1 else ENG)

    _legalize_waits(nc)
    return nc


# ---------------------------------------------------------------- driver
def _make_consts(inputs, states, qMax, Ro, tDiffusion,
                 Wp1, bp1, Wp2, bp2, Wp3, bp3, Wn, bn):
    tD = float(np.asarray(tDiffusion))
    c1 = DT / (VOLB * tD)
    c2 = DT / (VOLS * tD)
    const_q = bool(np.all(qMax == qMax.flat[0]))
    const_ro = bool(np.all(Ro == Ro.flat[0]))
    if const_q:
        qsm = float(qMax.flat[0]) * QMAX_BASE * VOLS / VOL
        rqs = 1.0 / qsm
    else:
        qsm = None
        rqs = None
    kro = float(Ro.flat[0]) * RO_BASE * DT / TO if const_ro else None

    # range of qpS_new for the poly fit (host, cheap)
    i = np.asarray(inputs, np.float64)[:, 0]
    qpB = np.asarray(states, np.float64)[:, 6]
    qpS = np.asarray(states, np.float64)[:, 7]
    qdot = (qpB / VOLB - qpS / VOLS) / tD
    qp_new = qpS + (i + qdot) * DT
    qlo, qhi = float(qp_new.min()), float(qp_new.max())
    margin = max(2.0, 0.25 * (qhi - qlo))
    qlo, qhi = qlo - margin, qhi + margin

    W1 = np.asarray(Wp1, np.float64)
    b1 = np.asarray(bp1, np.float64)
    W2 = np.asarray(Wp2, np.float64)
    b2 = np.asarray(bp2, np.float64)
    W3 = np.asarray(Wp3, np.float64)
    b3 = np.asarray(bp3, np.float64)
    if const_q:
        mid, hw, pc, fit_err = _fit_mlp_poly(qlo, qhi, rqs, W1, b1, W2, b2,
                                             W3, b3)
    else:
        # fit in x-domain over the widest plausible x range
        qsm_all = np.asarray(qMax, np.float64) * 1000.0
        x_new = qp_new / qsm_all
        xlo, xhi = float(x_new.min()), float(x_new.max())
        m = max(0.02, 0.25 * (xhi - xlo))
        xlo, xhi = xlo - m, xhi + m
        mid, hw, pc, fit_err = _fit_mlp_poly(xlo, xhi, 1.0, W1, b1, W2, b2,
                                             W3, b3)

    wn = float(np.asarray(Wn)[0, 0])
    bnv = float(np.asarray(bn)[0])
    wn_eff = wn * rqs if const_q else wn
    k0 = float(pc[0]) + U0P - U0N - bnv

    consts = dict(rqs=rqs, kro=kro, c1=c1, c2=c2,
                  poly=(mid, hw, pc), wn_eff=wn_eff, k0=k0,
                  fit_err=fit_err)
    return consts, (not const_q), (not const_ro)


def kernel(inputs, states, qMax, Ro, tDiffusion,
           Wp1, bp1, Wp2, bp2, Wp3, bp3, Wn, bn,
           _profile=False, _ntot=NTOT, _fd=FD):
    inputs = np.ascontiguousarray(np.asarray(inputs, np.float32))
    states = np.ascontiguousarray(np.asarray(states, np.float32))
    qMax = np.asarray(qMax, np.float32)
    Ro = np.asarray(Ro, np.float32)
    bc = P * _ntot
    assert inputs.shape[0] == NCORES * bc, (inputs.shape, _ntot)

    consts, general_q, general_ro = _make_consts(
        inputs, states, qMax, Ro, tDiffusion,
        Wp1, bp1, Wp2, bp2, Wp3, bp3, Wn, bn)

    sizes = TILE_SIZES if _ntot == NTOT else None
    nc = build_kernel(consts, fd=_fd, ntot=_ntot,
                      general_q=general_q, general_ro=general_ro,
                      it_bufs=3, xo_bufs=3, tile_sizes=sizes)

    in_maps = []
    for c in range(NCORES):
        sl = slice(c * bc, (c + 1) * bc)
        m = {"states": states[sl], "inputs": inputs[sl]}
        if general_q:
            m["qMax"] = np.ascontiguousarray(qMax[sl])
        if general_ro:
            m["Ro"] = np.ascontiguousarray(Ro[sl])
        in_maps.append(m)

    res = run_bass_kernel_spmd(nc, in_maps, core_ids=list(range(NCORES)))
    V = np.concatenate([res.results[c]["V"] for c in range(NCORES)], axis=0)
    Xnew = np.concatenate([res.results[c]["Xnew"] for c in range(NCORES)],
                          axis=0)
    kernel.last_nc = nc
    kernel.last_results = res
    return V, Xnew


# revision 39
# speedup vs baseline: 1.0060x; 1.0060x over previous
"""Trainium2 Bass kernel for the BatteryCell elementwise model.

Pure data-parallel over the batch: the 2,097,152-element batch is split
across 8 NeuronCores (262,144 each). Each core sees its shard as
[128 partitions x 2048 free] and streams it in tiles of FD columns.

Layout: `states` [Bc, 8] is DMA'd interleaved (contiguous per partition)
and the 8 state planes are accessed in place with stride-8 access
patterns; the two electrodes (n at e=5, p at e=7 etc.) are processed as
[128, FD, 2] "pair" APs so each ACT/DVE instruction covers both.

All transcendentals use only Ln / Exp / Square / Copy so the scalar
engine stays on one activation table the whole kernel (no table reloads):
  u^-1/2      = exp(-0.5*(ln q + ln(1 - q/qSMax)))   (no reciprocal op)
  arcsinh(z)  = ln(z + exp(0.5*ln(1 + z^2)))
  ln((1-x)/x) = ln(1-x) - ln(x)
The small MLPp (1->8->4->1 tanh net, a scalar function of xpS2 once its
weights are fixed) is evaluated as a build-time-fitted polynomial in
qpS_new with fused scalar_tensor_tensor Horner steps on the vector
engine. Inputs that are constant arrays in practice (qMax, Ro) and the
scalar tDiffusion plus the MLP weights are folded into immediates at
build time (the build is re-specialized per call from the actual numpy
values, with a general per-element fallback path when they are not
constant).
"""

import math

import numpy as np

import concourse.bass as bass
import concourse.mybir as mybir
from concourse import tile
from concourse.bass_utils import run_bass_kernel_spmd

# ---------------------------------------------------------------- constants
R_GAS = 8.3144621
F = 96487.0
ALPHA = 0.5
VOL = 2.2e-5
VOLS = 0.1 * VOL
VOLB = VOL - VOLS
SN = 2e-4
SP = 2e-4
KN = 2e4
KP = 2e4
TO = 10.0
TSN = 90.0
TSP = 90.0
U0P = 4.03
U0N = 0.01
QMAX_BASE = 10000.0
RO_BASE = 10.0
DT = 1.0

B = 2097152
NCORES = 8
BC = B // NCORES          # 262144 per core
P = 128
NTOT = BC // P            # 2048 free elements per partition
FD = 512                  # free elements per tile
NT = NTOT // FD           # tiles per core

f32 = mybir.dt.float32
AF = mybir.ActivationFunctionType
OP = mybir.AluOpType

# engine assignment for the TT-class ops that can move between engines.
# 'V' = vector (DVE), 'G' = gpsimd. Tuned from profiling.
ENG = {
    "qB_new": "G",
    "tb_copy": "G",
    "dd": "G",
    "d3": "V",
    "u1": "G",
    "u2": "G",
    "v2": "G",
    "hn": "V",
    "hp": "V",
    "vfin": "G",
    "qsn": "V",
    "lu": "G",
    "zz": "V",
    "ss": "V",
    "vo_new": "V",
}

# engine map for the final tile's epilogue: everything on the fast
# engines (DVE), since they are idle during the tail
ENG_TAIL = {k: ("V" if v == "G" else v) for k, v in ENG.items()}
ENG_TAIL["tb_copy"] = "G"
ENG_TAIL["lu"] = "G"

# head tile uses the steady-state map (measured best)
ENG_HEAD = dict(ENG)

# head/tail-tapered tile sizes (sum = NTOT) chosen via cost-model sweeps
TILE_SIZES = [96, 256, 512, 512, 512, 160]

_EVSEM_CAP = 2
_NO_SLOT = {"InstDrain", "InstNoOp"}


def _legalize_waits(nc, default_cap=1, evsem_cap=_EVSEM_CAP):
    """walrus cannot encode >cap sem waits on one instruction (and none at
    all on Drain/NoOp). Hoist excess waits onto InstEventSemaphore
    instructions inserted before the offender on the same engine."""
    nid = [0]
    for fn in nc.m.functions:
        for bb in fn.blocks:
            insts = bb.instructions
            i = 0
            while i < len(insts):
                ins = insts[i]
                si = ins.sync_info
                waits = list(si.on_wait) if si and si.on_wait else []
                cls = ins.__class__.__name__
                if cls in _NO_SLOT:
                    cap = 0
                elif cls == "InstEventSemaphore":
                    cap = evsem_cap
                else:
                    cap = default_cap
                if len(waits) > cap:
                    keep = waits[len(waits) - cap:] if cap else []
                    extra = waits[:len(waits) - cap] if cap else waits
                    new = []
                    for j in range(0, len(extra), evsem_cap):
                        ev = mybir.InstEventSemaphore(
                            name=f"WLG-{nid[0]}", ins=[], outs=[])
                        nid[0] += 1
                        ev.engine = ins.engine
                        ev.sync_info = mybir.SyncInfo(
                            on_wait=extra[j:j + evsem_cap], on_update=[])
                        new.append(ev)
                    ins.sync_info = mybir.SyncInfo(
                        on_wait=keep,
                        on_update=list(si.on_update) if si.on_update else [])
                    for k, ev in enumerate(new):
                        insts.insert(i + k, ev)
                    i += len(new)
                i += 1


# ------------------------------------------------------------- MLP poly fit
def _mlp_f(x, Wp1, bp1, Wp2, bp2, Wp3, bp3):
    """The exact MLPp as a scalar function of x (float64 numpy)."""
    h = np.tanh(np.outer(x, Wp1[0]) + bp1)
    h = np.tanh(h @ Wp2 + bp2)
    return (h @ Wp3 + bp3)[:, 0]


def _fit_mlp_poly(qlo, qhi, rqs, Wp1, bp1, Wp2, bp2, Wp3, bp3,
                  tol=2e-7, max_deg=14):
    """Fit F(q) = MLPp(rqs*q) over q in [qlo, qhi] as a polynomial in
    t = (q - mid)/hw. Returns (mid, hw, coeffs c_0..c_D)."""
    mid = 0.5 * (qlo + qhi)
    hw = 0.5 * (qhi - qlo)
    t = np.cos(np.pi * (np.arange(2000) + 0.5) / 2000)  # cheb nodes in [-1,1]
    q = mid + hw * t
    y = _mlp_f(q * rqs, Wp1, bp1, Wp2, bp2, Wp3, bp3)
    for deg in range(3, max_deg + 1):
        c = np.polynomial.chebyshev.chebfit(t, y, deg)
        err = np.abs(np.polynomial.chebyshev.chebval(t, c) - y).max()
        if err < tol:
            break
    pow_c = np.polynomial.chebyshev.cheb2poly(c)  # c_0..c_D in power basis
    return mid, hw, pow_c, err


# ---------------------------------------------------------------- builder
def build_kernel(consts, fd=FD, ntot=NTOT, general_q=False, general_ro=False,
                 im_bufs=1, st_bufs=3, it_bufs=3, xo_bufs=2, vo_bufs=2,
                 tile_sizes=None, b_lag=1):
    """Build the Bass module for one core's shard ([128, ntot] per plane).

    consts: dict with rqs (1/qSMax) or None, kro (Ro*RO_BASE*DT/TO) or
    None, c1, c2 (charge-diffusion constants), poly (mid, hw, coeffs),
    wn_eff (Wn*rqs), k0 (constant fold).
    """
    nt = ntot // fd
    nc = bass.Bass(trn_type="TRN2")

    st_d = nc.dram_tensor("states", [P * ntot, 8], f32, kind="ExternalInput")
    it_d = nc.dram_tensor("inputs", [P * ntot, 1], f32, kind="ExternalInput")
    if general_q:
        qm_d = nc.dram_tensor("qMax", [P * ntot], f32, kind="ExternalInput")
    if general_ro:
        ro_d = nc.dram_tensor("Ro", [P * ntot], f32, kind="ExternalInput")
    xo_d = nc.dram_tensor("Xnew", [P * ntot, 8], f32, kind="ExternalOutput")
    vo_d = nc.dram_tensor("V", [P * ntot, 1], f32, kind="ExternalOutput")

    st_v = st_d.rearrange("(p n) e -> p n e", p=P)
    it_v = it_d.rearrange("(p n) e -> p (n e)", p=P)
    xo_v = xo_d.rearrange("(p n) e -> p n e", p=P)
    vo_v = vo_d.rearrange("(p n) e -> p (n e)", p=P)
    if general_q:
        qm_v = qm_d.rearrange("(p n) -> p n", p=P)
    if general_ro:
        ro_v = ro_d.rearrange("(p n) -> p n", p=P)

    rqs = consts.get("rqs")
    kro = consts.get("kro")
    c1, c2 = consts["c1"], consts["c2"]
    mid, hw, pc = consts["poly"]
    wn_eff = consts["wn_eff"]
    k0 = consts["k0"]
    C1 = R_GAS / (F * ALPHA)
    LN8TH_N = math.log(1.0 / (2.0 * SN * KN))
    LN8TH_P = math.log(1.0 / (2.0 * SP * KP))
    assert abs(LN8TH_N - LN8TH_P) < 1e-12  # same fold for the n/p pair

    deg = len(pc) - 1
    # reversed-Horner params: G1 = A1*t + A0; G_{j+1} = (G_j + A_{j+1})*t
    # covers all degrees >= 1; constant pc[0] is folded into k0 upstream.
    A = [0.0] * (deg + 1)
    A[1] = float(pc[deg])
    A[0] = float(pc[deg - 1])
    for j in range(2, deg):
        A[j + 1] = float(pc[deg - j])
    # (A[2] stays 0; last A consumed at j = deg-1 -> A[deg] = pc[1])

    with tile.TileContext(nc) as tc:
        with (
            tc.tile_pool(name="io_st", bufs=st_bufs) as p_st,
            tc.tile_pool(name="io_it", bufs=it_bufs) as p_it,
            tc.tile_pool(name="io_xo", bufs=xo_bufs) as p_xo,
            tc.tile_pool(name="io_vo", bufs=vo_bufs) as p_vo,
            tc.tile_pool(name="im", bufs=im_bufs) as im,
            tc.tile_pool(name="imq", bufs=max(2, im_bufs)) as imq,
            tc.tile_pool(name="cst", bufs=1) as cst,
        ):
            ln8 = cst.tile([P, 1], f32, tag="ln8")
            nc.gpsimd.memset(ln8[:], LN8TH_N)

            eng = {"V": nc.vector, "G": nc.gpsimd}

            def stage_a(t, off, fdt, eng_map=ENG):
                """DMA-in + kinetics + state updates; writes all of Xnew."""
                sl = slice(off, off + fdt)
                st = p_st.tile([P, fdt * 8], f32, tag="st", name="st")
                nc.sync.dma_start(
                    st.rearrange("p (n e) -> p n e", e=8), st_v[:, sl, :])
                it = p_it.tile([P, fdt], f32, tag="it", name="it")
                nc.sync.dma_start(it, it_v[:, sl])
                qm = ro = None
                if general_q:
                    qm = p_it.tile([P, fdt], f32, tag="qm", name="qm")
                    nc.sync.dma_start(qm, qm_v[:, sl])
                if general_ro:
                    ro = p_it.tile([P, fdt], f32, tag="ro", name="ro")
                    nc.sync.dma_start(ro, ro_v[:, sl])
                xo = p_xo.tile([P, fdt * 8], f32, tag="xo", name="xo")

                s3 = st.rearrange("p (n e) -> p n e", e=8)
                x3 = xo.rearrange("p (n e) -> p n e", e=8)
                sS = s3[:, :, 5:8:2]     # (qnS, qpS) pair
                sB = s3[:, :, 4:8:2]     # (qnB, qpB) pair
                itb = it.broadcast_to((P, fdt, 2))

                def pair(tag):
                    tl = im.tile([P, fdt * 2], f32, tag=tag, name=tag)
                    return tl.rearrange("p (n e) -> p n e", e=2)

                def plane(tag):
                    return im.tile([P, fdt], f32, tag=tag, name=tag)

                lnqs_b = qs_b = lnqs = None
                if general_q:
                    lnqs = plane("lnqs")
                    nc.scalar.activation(lnqs, qm, AF.Ln, scale=1000.0)
                    lnqs_b = lnqs.broadcast_to((P, fdt, 2))

                # ---- electrode kinetics on old states (paired n/p) ----
                l1 = pair("l1")
                l2 = pair("l2")
                if general_q:
                    qs = plane("qs")
                    nc.scalar.activation(qs, qm, AF.Copy, scale=1000.0)
                    qs_b = qs.broadcast_to((P, fdt, 2))
                    dq = pair("dq")
                    nc.vector.tensor_sub(dq, qs_b, sS)
                    nc.scalar.activation(l1, sS, AF.Ln)
                    nc.scalar.activation(l2, dq, AF.Ln)
                    lu0 = pair("lu0")
                    nc.vector.tensor_add(lu0, l1, l2)
                    lu = pair("lu")
                    nc.vector.scalar_tensor_tensor(
                        lu, lnqs_b, -2.0, lu0, OP.mult, OP.add)
                else:
                    nc.scalar.activation(l1, sS, AF.Ln, scale=rqs)
                    nc.scalar.activation(l2, sS, AF.Ln, scale=-rqs, bias=1.0)
                    lu = pair("lu")
                    eng[eng_map["lu"]].tensor_add(lu, l1, l2)
                ee = pair("ee")
                nc.scalar.activation(ee, lu, AF.Exp, scale=-0.5, bias=ln8[:])
                zz = pair("zz")
                eng[eng_map["zz"]].tensor_mul(zz, ee, itb)
                sq = pair("sq")
                nc.scalar.activation(sq, zz, AF.Square)
                lz = pair("lz")
                nc.scalar.activation(lz, sq, AF.Ln, bias=1.0)
                ww = pair("ww")
                nc.scalar.activation(ww, lz, AF.Exp, scale=0.5)
                ss = pair("ss")
                eng[eng_map["ss"]].tensor_add(ss, zz, ww)
                aa = pair("aa")                       # asinh pair (n, p)
                nc.scalar.activation(aa, ss, AF.Ln)

                # ---- V-state updates ----
                tbb = s3[:, :, 0].broadcast_to((P, fdt, 2))
                n90 = pair("n90")
                nc.vector.scalar_tensor_tensor(
                    n90, tbb, C1 / TSN, aa, OP.mult, OP.mult)  # VsNom/90
                nc.vector.scalar_tensor_tensor(
                    x3[:, :, 2:4], s3[:, :, 2:4], (TSN - 1.0) / TSN, n90,
                    OP.mult, OP.add)
                ta = plane("ta")
                if general_ro:
                    nc.vector.scalar_tensor_tensor(
                        ta, it, RO_BASE * DT / TO, ro, OP.mult, OP.mult)
                else:
                    nc.scalar.activation(ta, it, AF.Copy, scale=kro)
                if eng_map["vo_new"] == "G":
                    vo9 = plane("vo9")
                    nc.scalar.activation(vo9, s3[:, :, 1], AF.Copy,
                                         scale=1.0 - DT / TO)
                    nc.gpsimd.tensor_add(x3[:, :, 1], vo9, ta)
                else:
                    nc.vector.scalar_tensor_tensor(
                        x3[:, :, 1], s3[:, :, 1], 1.0 - DT / TO, ta,
                        OP.mult, OP.add)

                # ---- charge updates ----
                # qd2 = (c2/c1)*qS - qB = -qdot*DT/c1  (c2/c1 = VOLB/VOLS)
                qd = pair("qd")
                nc.vector.scalar_tensor_tensor(
                    qd, sS, c2 / c1, sB, OP.mult, OP.subtract)
                nc.vector.scalar_tensor_tensor(
                    x3[:, :, 4:8:2], qd, c1, sB, OP.mult, OP.add)
                hn = plane("hn")
                nc.vector.scalar_tensor_tensor(
                    hn, qd[:, :, 0], -c1, it, OP.mult, OP.subtract)
                hp = plane("hp")
                nc.vector.scalar_tensor_tensor(
                    hp, qd[:, :, 1], -c1, it, OP.mult, OP.add)
                eng[eng_map["qsn"]].tensor_add(x3[:, :, 5], s3[:, :, 5], hn)
                eng[eng_map["qsn"]].tensor_add(x3[:, :, 7], s3[:, :, 7], hp)
                eng[eng_map["tb_copy"]].tensor_copy(x3[:, :, 0], s3[:, :, 0])
                nc.sync.dma_start(
                    xo_v[:, sl, :], xo.rearrange("p (n e) -> p n e", e=8))
                return dict(sl=sl, fdt=fdt, xo=xo, x3=x3,
                            lnqs=lnqs, lnqs_b=lnqs_b, qs_b=qs_b)

            def stage_b(ctx, eng_map=ENG):
                """Output voltage from Xnew + DMA-out."""
                sl, fdt = ctx["sl"], ctx["fdt"]
                x3, xo = ctx["x3"], ctx["xo"]
                vo = p_vo.tile([P, fdt], f32, tag="vo", name="vo")

                def pair(tag):
                    tl = im.tile([P, fdt * 2], f32, tag=tag, name=tag)
                    return tl.rearrange("p (n e) -> p n e", e=2)

                def plane(tag):
                    return im.tile([P, fdt], f32, tag=tag, name=tag)

                xS = x3[:, :, 5:8:2]                    # (qnS_new, qpS_new)
                m1 = pair("m1")
                m2 = pair("m2")
                if general_q:
                    dq2 = pair("dq2")
                    nc.vector.tensor_sub(dq2, ctx["qs_b"], xS)
                    nc.scalar.activation(m1, xS, AF.Ln)
                    nc.scalar.activation(m2, dq2, AF.Ln)
                else:
                    nc.scalar.activation(m1, xS, AF.Ln, scale=rqs)
                    nc.scalar.activation(m2, xS, AF.Ln, scale=-rqs, bias=1.0)
                dd = pair("dd")
                eng[eng_map["dd"]].tensor_sub(dd, m2, m1)   # ln((1-x)/x) pair
                d3 = plane("d3")
                eng[eng_map["d3"]].tensor_sub(d3, dd[:, :, 1], dd[:, :, 0])
                vv = plane("vv")
                nc.vector.scalar_tensor_tensor(
                    vv, x3[:, :, 0], C1 * 0.5, d3, OP.mult, OP.mult)
                vvk = plane("vvk")
                nc.scalar.activation(vvk, vv, AF.Copy, bias=k0)

                # ---- MLP polynomial in qpS_new ----
                xx2 = None
                if general_q:
                    lnx2 = pair("lnx2")
                    nc.scalar.activation(lnx2, xS, AF.Ln)
                    xq2 = pair("xq2")
                    nc.vector.scalar_tensor_tensor(
                        xq2, ctx["lnqs_b"], -1.0, lnx2, OP.mult, OP.add)
                    xx2 = pair("xx2")
                    nc.scalar.activation(xx2, xq2, AF.Exp)  # x2 pair
                    tq = plane("tq")
                    nc.vector.tensor_scalar(
                        tq, xx2[:, :, 1], 1.0 / hw, -mid / hw,
                        OP.mult, OP.add)
                else:
                    tq = plane("tq")
                    nc.scalar.activation(tq, x3[:, :, 7], AF.Copy,
                                         scale=1.0 / hw, bias=-mid / hw)
                g = imq.tile([P, fdt], f32, tag="gpoly", name="gpoly")
                nc.vector.tensor_scalar(g, tq, A[1], A[0], OP.mult, OP.add)
                for j in range(2, deg + 1):
                    g2 = imq.tile([P, fdt], f32, tag=f"gpoly{j % 2}",
                                  name=f"gpoly{j % 2}")
                    nc.vector.scalar_tensor_tensor(
                        g2, g, A[j], tq, OP.add, OP.mult)
                    g = g2

                # ---- assemble V ----
                v1 = plane("v1")
                xn_src = xx2[:, :, 0] if general_q else x3[:, :, 5]
                nc.vector.scalar_tensor_tensor(
                    v1, xn_src, -wn_eff, g, OP.mult, OP.add)
                v2 = plane("v2")
                eng[eng_map["v2"]].tensor_add(v2, v1, vvk)
                u1 = plane("u1")
                eng[eng_map["u1"]].tensor_add(u1, x3[:, :, 1], x3[:, :, 2])
                u2 = plane("u2")
                eng[eng_map["u2"]].tensor_add(u2, u1, x3[:, :, 3])
                eng[eng_map["vfin"]].tensor_sub(vo, v2, u2)

                nc.sync.dma_start(vo_v[:, sl], vo)

            # 2-stage software pipeline over the tiles: stage_b of tile t
            # is emitted between stage_a of t+1 and t+2, so every engine's
            # in-order queue interleaves independent work.
            sizes = tile_sizes if tile_sizes is not None else [fd] * nt
            assert sum(sizes) == ntot
            ctxs = []
            off = 0
            nts = len(sizes)
            for i, fdt in enumerate(sizes):
                am = ENG_HEAD if i == 0 else ENG
                ctxs.append(stage_a(i, off, fdt, eng_map=am))
                off += fdt
                if i >= b_lag:
                    stage_b(ctxs[i - b_lag])
            for j in range(nts - b_lag, nts):
                stage_b(ctxs[j],
                        eng_map=ENG_TAIL if j == nts - 1 else ENG)

    _legalize_waits(nc)
    return nc


# ---------------------------------------------------------------- driver
def _make_consts(inputs, states, qMax, Ro, tDiffusion,
                 Wp1, bp1, Wp2, bp2, Wp3, bp3, Wn, bn):
    tD = float(np.asarray(tDiffusion))
    c1 = DT / (VOLB * tD)
    c2 = DT / (VOLS * tD)
    const_q = bool(np.all(qMax == qMax.flat[0]))
    const_ro = bool(np.all(Ro == Ro.flat[0]))
    if const_q:
        qsm = float(qMax.flat[0]) * QMAX_BASE * VOLS / VOL
        rqs = 1.0 / qsm
    else:
        qsm = None
        rqs = None
    kro = float(Ro.flat[0]) * RO_BASE * DT / TO if const_ro else None

    # range of qpS_new for the poly fit (host, cheap)
    i = np.asarray(inputs, np.float64)[:, 0]
    qpB = np.asarray(states, np.float64)[:, 6]
    qpS = np.asarray(states, np.float64)[:, 7]
    qdot = (qpB / VOLB - qpS / VOLS) / tD
    qp_new = qpS + (i + qdot) * DT
    qlo, qhi = float(qp_new.min()), float(qp_new.max())
    margin = max(2.0, 0.25 * (qhi - qlo))
    qlo, qhi = qlo - margin, qhi + margin

    W1 = np.asarray(Wp1, np.float64)
    b1 = np.asarray(bp1, np.float64)
    W2 = np.asarray(Wp2, np.float64)
    b2 = np.asarray(bp2, np.float64)
    W3 = np.asarray(Wp3, np.float64)
    b3 = np.asarray(bp3, np.float64)
    if const_q:
        mid, hw, pc, fit_err = _fit_mlp_poly(qlo, qhi, rqs, W1, b1, W2, b2,
                                             W3, b3)
    else:
        # fit in x-domain over the widest plausible x range
        qsm_all = np.asarray(qMax, np.float64) * 1000.0
        x_new = qp_new / qsm_all
        xlo, xhi = float(x_new.min()), float(x_new.max())
        m = max(0.02, 0.25 * (xhi - xlo))
        xlo, xhi = xlo - m, xhi + m
        mid, hw, pc, fit_err = _fit_mlp_poly(xlo, xhi, 1.0, W1, b1, W2, b2,
                                             W3, b3)

    wn = float(np.asarray(Wn)[0, 0])
    bnv = float(np.asarray(bn)[0])
    wn_eff = wn * rqs if const_q else wn
    k0 = float(pc[0]) + U0P - U0N - bnv

    consts = dict(rqs=rqs, kro=kro, c1=c1, c2=c2,
                  poly=(mid, hw, pc), wn_eff=wn_eff, k0=k0,
                  fit_err=fit_err)
    return consts, (not const_q), (not const_ro)


def kernel(inputs, states, qMax, Ro, tDiffusion,
           Wp1, bp1, Wp2, bp2, Wp3, bp3, Wn, bn,
           _profile=False, _ntot=NTOT, _fd=FD):
    inputs = np.ascontiguousarray(np.asarray(inputs, np.float32))
    states = np.ascontiguousarray(np.asarray(states, np.float32))
    qMax = np.asarray(qMax, np.float32)
    Ro = np.asarray(Ro, np.float32)
    bc = P * _ntot
    assert inputs.shape[0] == NCORES * bc, (inputs.shape, _ntot)

    consts, general_q, general_ro = _make_consts(
        inputs, states, qMax, Ro, tDiffusion,
        Wp1, bp1, Wp2, bp2, Wp3, bp3, Wn, bn)

    sizes = TILE_SIZES if _ntot == NTOT else None
    nc = build_kernel(consts, fd=_fd, ntot=_ntot,
                      general_q=general_q, general_ro=general_ro,
                      it_bufs=3, xo_bufs=3, tile_sizes=sizes)

    in_maps = []
    for c in range(NCORES):
        sl = slice(c * bc, (c + 1) * bc)
        m = {"states": states[sl], "inputs": inputs[sl]}
        if general_q:
            m["qMax"] = np.ascontiguousarray(qMax[sl])
        if general_ro:
            m["Ro"] = np.ascontiguousarray(Ro[sl])
        in_maps.append(m)

    res = run_bass_kernel_spmd(nc, in_maps, core_ids=list(range(NCORES)))
    V = np.concatenate([res.results[c]["V"] for c in range(NCORES)], axis=0)
    Xnew = np.concatenate([res.results[c]["Xnew"] for c in range(NCORES)],
                          axis=0)
    kernel.last_nc = nc
    kernel.last_results = res
    return V, Xnew


# revision 40
# speedup vs baseline: 1.0204x; 1.0143x over previous
"""Trainium2 Bass kernel for the BatteryCell elementwise model.

Pure data-parallel over the batch: the 2,097,152-element batch is split
across 8 NeuronCores (262,144 each). Each core sees its shard as
[128 partitions x 2048 free] and streams it in tiles of FD columns.

Layout: `states` [Bc, 8] is DMA'd interleaved (contiguous per partition)
and the 8 state planes are accessed in place with stride-8 access
patterns; the two electrodes (n at e=5, p at e=7 etc.) are processed as
[128, FD, 2] "pair" APs so each ACT/DVE instruction covers both.

All transcendentals use only Ln / Exp / Square / Copy so the scalar
engine stays on one activation table the whole kernel (no table reloads):
  u^-1/2      = exp(-0.5*(ln q + ln(1 - q/qSMax)))   (no reciprocal op)
  arcsinh(z)  = ln(z + exp(0.5*ln(1 + z^2)))
  ln((1-x)/x) = ln(1-x) - ln(x)
The small MLPp (1->8->4->1 tanh net, a scalar function of xpS2 once its
weights are fixed) is evaluated as a build-time-fitted polynomial in
qpS_new with fused scalar_tensor_tensor Horner steps on the vector
engine. Inputs that are constant arrays in practice (qMax, Ro) and the
scalar tDiffusion plus the MLP weights are folded into immediates at
build time (the build is re-specialized per call from the actual numpy
values, with a general per-element fallback path when they are not
constant).
"""

import math

import numpy as np

import concourse.bass as bass
import concourse.mybir as mybir
from concourse import tile
from concourse.bass_utils import run_bass_kernel_spmd

# ---------------------------------------------------------------- constants
R_GAS = 8.3144621
F = 96487.0
ALPHA = 0.5
VOL = 2.2e-5
VOLS = 0.1 * VOL
VOLB = VOL - VOLS
SN = 2e-4
SP = 2e-4
KN = 2e4
KP = 2e4
TO = 10.0
TSN = 90.0
TSP = 90.0
U0P = 4.03
U0N = 0.01
QMAX_BASE = 10000.0
RO_BASE = 10.0
DT = 1.0

B = 2097152
NCORES = 8
BC = B // NCORES          # 262144 per core
P = 128
NTOT = BC // P            # 2048 free elements per partition
FD = 512                  # free elements per tile
NT = NTOT // FD           # tiles per core

f32 = mybir.dt.float32
AF = mybir.ActivationFunctionType
OP = mybir.AluOpType

# engine assignment for the TT-class ops that can move between engines.
# 'V' = vector (DVE), 'G' = gpsimd. Tuned from profiling.
ENG = {
    "qB_new": "G",
    "tb_copy": "G",
    "dd": "G",
    "d3": "V",
    "u1": "G",
    "u2": "G",
    "v2": "G",
    "hn": "V",
    "hp": "V",
    "vfin": "G",
    "qsn": "V",
    "lu": "G",
    "zz": "V",
    "ss": "V",
    "vo_new": "V",
}

# engine map for the final tile's epilogue: everything on the fast
# engines (DVE), since they are idle during the tail
ENG_TAIL = {k: ("V" if v == "G" else v) for k, v in ENG.items()}
ENG_TAIL["tb_copy"] = "G"
ENG_TAIL["lu"] = "G"

# head tile uses the steady-state map (measured best)
ENG_HEAD = dict(ENG)

# head/tail-tapered tile sizes (sum = NTOT) chosen via cost-model sweeps
TILE_SIZES = [192, 288, 512, 512, 384, 160]

_EVSEM_CAP = 2
_NO_SLOT = {"InstDrain", "InstNoOp"}


def _legalize_waits(nc, default_cap=1, evsem_cap=_EVSEM_CAP):
    """walrus cannot encode >cap sem waits on one instruction (and none at
    all on Drain/NoOp). Hoist excess waits onto InstEventSemaphore
    instructions inserted before the offender on the same engine."""
    nid = [0]
    for fn in nc.m.functions:
        for bb in fn.blocks:
            insts = bb.instructions
            i = 0
            while i < len(insts):
                ins = insts[i]
                si = ins.sync_info
                waits = list(si.on_wait) if si and si.on_wait else []
                cls = ins.__class__.__name__
                if cls in _NO_SLOT:
                    cap = 0
                elif cls == "InstEventSemaphore":
                    cap = evsem_cap
                else:
                    cap = default_cap
                if len(waits) > cap:
                    keep = waits[len(waits) - cap:] if cap else []
                    extra = waits[:len(waits) - cap] if cap else waits
                    new = []
                    for j in range(0, len(extra), evsem_cap):
                        ev = mybir.InstEventSemaphore(
                            name=f"WLG-{nid[0]}", ins=[], outs=[])
                        nid[0] += 1
                        ev.engine = ins.engine
                        ev.sync_info = mybir.SyncInfo(
                            on_wait=extra[j:j + evsem_cap], on_update=[])
                        new.append(ev)
                    ins.sync_info = mybir.SyncInfo(
                        on_wait=keep,
                        on_update=list(si.on_update) if si.on_update else [])
                    for k, ev in enumerate(new):
                        insts.insert(i + k, ev)
                    i += len(new)
                i += 1


# ------------------------------------------------------------- MLP poly fit
def _mlp_f(x, Wp1, bp1, Wp2, bp2, Wp3, bp3):
    """The exact MLPp as a scalar function of x (float64 numpy)."""
    h = np.tanh(np.outer(x, Wp1[0]) + bp1)
    h = np.tanh(h @ Wp2 + bp2)
    return (h @ Wp3 + bp3)[:, 0]


def _fit_mlp_poly(qlo, qhi, rqs, Wp1, bp1, Wp2, bp2, Wp3, bp3,
                  tol=2e-7, max_deg=14):
    """Fit F(q) = MLPp(rqs*q) over q in [qlo, qhi] as a polynomial in
    t = (q - mid)/hw. Returns (mid, hw, coeffs c_0..c_D)."""
    mid = 0.5 * (qlo + qhi)
    hw = 0.5 * (qhi - qlo)
    t = np.cos(np.pi * (np.arange(2000) + 0.5) / 2000)  # cheb nodes in [-1,1]
    q = mid + hw * t
    y = _mlp_f(q * rqs, Wp1, bp1, Wp2, bp2, Wp3, bp3)
    for deg in range(3, max_deg + 1):
        c = np.polynomial.chebyshev.chebfit(t, y, deg)
        err = np.abs(np.polynomial.chebyshev.chebval(t, c) - y).max()
        if err < tol:
            break
    pow_c = np.polynomial.chebyshev.cheb2poly(c)  # c_0..c_D in power basis
    return mid, hw, pow_c, err


# ---------------------------------------------------------------- builder
def build_kernel(consts, fd=FD, ntot=NTOT, general_q=False, general_ro=False,
                 im_bufs=1, st_bufs=3, it_bufs=3, xo_bufs=2, vo_bufs=2,
                 tile_sizes=None, b_lag=1):
    """Build the Bass module for one core's shard ([128, ntot] per plane).

    consts: dict with rqs (1/qSMax) or None, kro (Ro*RO_BASE*DT/TO) or
    None, c1, c2 (charge-diffusion constants), poly (mid, hw, coeffs),
    wn_eff (Wn*rqs), k0 (constant fold).
    """
    nt = ntot // fd
    nc = bass.Bass(trn_type="TRN2")

    st_d = nc.dram_tensor("states", [P * ntot, 8], f32, kind="ExternalInput")
    it_d = nc.dram_tensor("inputs", [P * ntot, 1], f32, kind="ExternalInput")
    if general_q:
        qm_d = nc.dram_tensor("qMax", [P * ntot], f32, kind="ExternalInput")
    if general_ro:
        ro_d = nc.dram_tensor("Ro", [P * ntot], f32, kind="ExternalInput")
    xo_d = nc.dram_tensor("Xnew", [P * ntot, 8], f32, kind="ExternalOutput")
    vo_d = nc.dram_tensor("V", [P * ntot, 1], f32, kind="ExternalOutput")

    st_v = st_d.rearrange("(p n) e -> p n e", p=P)
    it_v = it_d.rearrange("(p n) e -> p (n e)", p=P)
    xo_v = xo_d.rearrange("(p n) e -> p n e", p=P)
    vo_v = vo_d.rearrange("(p n) e -> p (n e)", p=P)
    if general_q:
        qm_v = qm_d.rearrange("(p n) -> p n", p=P)
    if general_ro:
        ro_v = ro_d.rearrange("(p n) -> p n", p=P)

    rqs = consts.get("rqs")
    kro = consts.get("kro")
    c1, c2 = consts["c1"], consts["c2"]
    mid, hw, pc = consts["poly"]
    wn_eff = consts["wn_eff"]
    k0 = consts["k0"]
    C1 = R_GAS / (F * ALPHA)
    LN8TH_N = math.log(1.0 / (2.0 * SN * KN))
    LN8TH_P = math.log(1.0 / (2.0 * SP * KP))
    assert abs(LN8TH_N - LN8TH_P) < 1e-12  # same fold for the n/p pair

    deg = len(pc) - 1
    # reversed-Horner params: G1 = A1*t + A0; G_{j+1} = (G_j + A_{j+1})*t
    # covers all degrees >= 1; constant pc[0] is folded into k0 upstream.
    A = [0.0] * (deg + 1)
    A[1] = float(pc[deg])
    A[0] = float(pc[deg - 1])
    for j in range(2, deg):
        A[j + 1] = float(pc[deg - j])
    # (A[2] stays 0; last A consumed at j = deg-1 -> A[deg] = pc[1])

    with tile.TileContext(nc) as tc:
        with (
            tc.tile_pool(name="io_st", bufs=st_bufs) as p_st,
            tc.tile_pool(name="io_it", bufs=it_bufs) as p_it,
            tc.tile_pool(name="io_xo", bufs=xo_bufs) as p_xo,
            tc.tile_pool(name="io_vo", bufs=vo_bufs) as p_vo,
            tc.tile_pool(name="im", bufs=im_bufs) as im,
            tc.tile_pool(name="imq", bufs=max(2, im_bufs)) as imq,
            tc.tile_pool(name="cst", bufs=1) as cst,
        ):
            ln8 = cst.tile([P, 1], f32, tag="ln8")
            nc.gpsimd.memset(ln8[:], LN8TH_N)

            eng = {"V": nc.vector, "G": nc.gpsimd}

            def stage_a(t, off, fdt, eng_map=ENG):
                """DMA-in + kinetics + state updates; writes all of Xnew."""
                sl = slice(off, off + fdt)
                st = p_st.tile([P, fdt * 8], f32, tag="st", name="st")
                nc.sync.dma_start(
                    st.rearrange("p (n e) -> p n e", e=8), st_v[:, sl, :])
                it = p_it.tile([P, fdt], f32, tag="it", name="it")
                nc.sync.dma_start(it, it_v[:, sl])
                qm = ro = None
                if general_q:
                    qm = p_it.tile([P, fdt], f32, tag="qm", name="qm")
                    nc.sync.dma_start(qm, qm_v[:, sl])
                if general_ro:
                    ro = p_it.tile([P, fdt], f32, tag="ro", name="ro")
                    nc.sync.dma_start(ro, ro_v[:, sl])
                xo = p_xo.tile([P, fdt * 8], f32, tag="xo", name="xo")

                s3 = st.rearrange("p (n e) -> p n e", e=8)
                x3 = xo.rearrange("p (n e) -> p n e", e=8)
                sS = s3[:, :, 5:8:2]     # (qnS, qpS) pair
                sB = s3[:, :, 4:8:2]     # (qnB, qpB) pair
                itb = it.broadcast_to((P, fdt, 2))

                def pair(tag):
                    tl = im.tile([P, fdt * 2], f32, tag=tag, name=tag)
                    return tl.rearrange("p (n e) -> p n e", e=2)

                def plane(tag):
                    return im.tile([P, fdt], f32, tag=tag, name=tag)

                lnqs_b = qs_b = lnqs = None
                if general_q:
                    lnqs = plane("lnqs")
                    nc.scalar.activation(lnqs, qm, AF.Ln, scale=1000.0)
                    lnqs_b = lnqs.broadcast_to((P, fdt, 2))

                # ---- electrode kinetics on old states (paired n/p) ----
                l1 = pair("l1")
                l2 = pair("l2")
                if general_q:
                    qs = plane("qs")
                    nc.scalar.activation(qs, qm, AF.Copy, scale=1000.0)
                    qs_b = qs.broadcast_to((P, fdt, 2))
                    dq = pair("dq")
                    nc.vector.tensor_sub(dq, qs_b, sS)
                    nc.scalar.activation(l1, sS, AF.Ln)
                    nc.scalar.activation(l2, dq, AF.Ln)
                    lu0 = pair("lu0")
                    nc.vector.tensor_add(lu0, l1, l2)
                    lu = pair("lu")
                    nc.vector.scalar_tensor_tensor(
                        lu, lnqs_b, -2.0, lu0, OP.mult, OP.add)
                else:
                    nc.scalar.activation(l1, sS, AF.Ln, scale=rqs)
                    nc.scalar.activation(l2, sS, AF.Ln, scale=-rqs, bias=1.0)
                    lu = pair("lu")
                    eng[eng_map["lu"]].tensor_add(lu, l1, l2)
                ee = pair("ee")
                nc.scalar.activation(ee, lu, AF.Exp, scale=-0.5, bias=ln8[:])
                zz = pair("zz")
                eng[eng_map["zz"]].tensor_mul(zz, ee, itb)
                sq = pair("sq")
                nc.scalar.activation(sq, zz, AF.Square)
                lz = pair("lz")
                nc.scalar.activation(lz, sq, AF.Ln, bias=1.0)
                ww = pair("ww")
                nc.scalar.activation(ww, lz, AF.Exp, scale=0.5)
                ss = pair("ss")
                eng[eng_map["ss"]].tensor_add(ss, zz, ww)
                aa = pair("aa")                       # asinh pair (n, p)
                nc.scalar.activation(aa, ss, AF.Ln)

                # ---- V-state updates ----
                tbb = s3[:, :, 0].broadcast_to((P, fdt, 2))
                n90 = pair("n90")
                nc.vector.scalar_tensor_tensor(
                    n90, tbb, C1 / TSN, aa, OP.mult, OP.mult)  # VsNom/90
                nc.vector.scalar_tensor_tensor(
                    x3[:, :, 2:4], s3[:, :, 2:4], (TSN - 1.0) / TSN, n90,
                    OP.mult, OP.add)
                ta = plane("ta")
                if general_ro:
                    nc.vector.scalar_tensor_tensor(
                        ta, it, RO_BASE * DT / TO, ro, OP.mult, OP.mult)
                else:
                    nc.scalar.activation(ta, it, AF.Copy, scale=kro)
                if eng_map["vo_new"] == "G":
                    vo9 = plane("vo9")
                    nc.scalar.activation(vo9, s3[:, :, 1], AF.Copy,
                                         scale=1.0 - DT / TO)
                    nc.gpsimd.tensor_add(x3[:, :, 1], vo9, ta)
                else:
                    nc.vector.scalar_tensor_tensor(
                        x3[:, :, 1], s3[:, :, 1], 1.0 - DT / TO, ta,
                        OP.mult, OP.add)

                # ---- charge updates ----
                # qd2 = (c2/c1)*qS - qB = -qdot*DT/c1  (c2/c1 = VOLB/VOLS)
                qd = pair("qd")
                nc.vector.scalar_tensor_tensor(
                    qd, sS, c2 / c1, sB, OP.mult, OP.subtract)
                nc.vector.scalar_tensor_tensor(
                    x3[:, :, 4:8:2], qd, c1, sB, OP.mult, OP.add)
                hn = plane("hn")
                nc.vector.scalar_tensor_tensor(
                    hn, qd[:, :, 0], -c1, it, OP.mult, OP.subtract)
                hp = plane("hp")
                nc.vector.scalar_tensor_tensor(
                    hp, qd[:, :, 1], -c1, it, OP.mult, OP.add)
                eng[eng_map["qsn"]].tensor_add(x3[:, :, 5], s3[:, :, 5], hn)
                eng[eng_map["qsn"]].tensor_add(x3[:, :, 7], s3[:, :, 7], hp)
                eng[eng_map["tb_copy"]].tensor_copy(x3[:, :, 0], s3[:, :, 0])
                nc.sync.dma_start(
                    xo_v[:, sl, :], xo.rearrange("p (n e) -> p n e", e=8))
                return dict(sl=sl, fdt=fdt, xo=xo, x3=x3,
                            lnqs=lnqs, lnqs_b=lnqs_b, qs_b=qs_b)

            def stage_b(ctx, eng_map=ENG):
                """Output voltage from Xnew + DMA-out."""
                sl, fdt = ctx["sl"], ctx["fdt"]
                x3, xo = ctx["x3"], ctx["xo"]
                vo = p_vo.tile([P, fdt], f32, tag="vo", name="vo")

                def pair(tag):
                    tl = im.tile([P, fdt * 2], f32, tag=tag, name=tag)
                    return tl.rearrange("p (n e) -> p n e", e=2)

                def plane(tag):
                    return im.tile([P, fdt], f32, tag=tag, name=tag)

                xS = x3[:, :, 5:8:2]                    # (qnS_new, qpS_new)
                m1 = pair("m1")
                m2 = pair("m2")
                if general_q:
                    dq2 = pair("dq2")
                    nc.vector.tensor_sub(dq2, ctx["qs_b"], xS)
                    nc.scalar.activation(m1, xS, AF.Ln)
                    nc.scalar.activation(m2, dq2, AF.Ln)
                else:
                    nc.scalar.activation(m1, xS, AF.Ln, scale=rqs)
                    nc.scalar.activation(m2, xS, AF.Ln, scale=-rqs, bias=1.0)
                dd = pair("dd")
                eng[eng_map["dd"]].tensor_sub(dd, m2, m1)   # ln((1-x)/x) pair
                d3 = plane("d3")
                eng[eng_map["d3"]].tensor_sub(d3, dd[:, :, 1], dd[:, :, 0])
                vv = plane("vv")
                nc.vector.scalar_tensor_tensor(
                    vv, x3[:, :, 0], C1 * 0.5, d3, OP.mult, OP.mult)
                vvk = plane("vvk")
                nc.scalar.activation(vvk, vv, AF.Copy, bias=k0)

                # ---- MLP polynomial in qpS_new ----
                xx2 = None
                if general_q:
                    lnx2 = pair("lnx2")
                    nc.scalar.activation(lnx2, xS, AF.Ln)
                    xq2 = pair("xq2")
                    nc.vector.scalar_tensor_tensor(
                        xq2, ctx["lnqs_b"], -1.0, lnx2, OP.mult, OP.add)
                    xx2 = pair("xx2")
                    nc.scalar.activation(xx2, xq2, AF.Exp)  # x2 pair
                    tq = plane("tq")
                    nc.vector.tensor_scalar(
                        tq, xx2[:, :, 1], 1.0 / hw, -mid / hw,
                        OP.mult, OP.add)
                else:
                    tq = plane("tq")
                    nc.scalar.activation(tq, x3[:, :, 7], AF.Copy,
                                         scale=1.0 / hw, bias=-mid / hw)
                g = imq.tile([P, fdt], f32, tag="gpoly", name="gpoly")
                nc.vector.tensor_scalar(g, tq, A[1], A[0], OP.mult, OP.add)
                for j in range(2, deg + 1):
                    g2 = imq.tile([P, fdt], f32, tag=f"gpoly{j % 2}",
                                  name=f"gpoly{j % 2}")
                    nc.vector.scalar_tensor_tensor(
                        g2, g, A[j], tq, OP.add, OP.mult)
                    g = g2

                # ---- assemble V ----
                v1 = plane("v1")
                xn_src = xx2[:, :, 0] if general_q else x3[:, :, 5]
                nc.vector.scalar_tensor_tensor(
                    v1, xn_src, -wn_eff, g, OP.mult, OP.add)
                v2 = plane("v2")
                eng[eng_map["v2"]].tensor_add(v2, v1, vvk)
                u1 = plane("u1")
                eng[eng_map["u1"]].tensor_add(u1, x3[:, :, 1], x3[:, :, 2])
                u2 = plane("u2")
                eng[eng_map["u2"]].tensor_add(u2, u1, x3[:, :, 3])
                eng[eng_map["vfin"]].tensor_sub(vo, v2, u2)

                nc.sync.dma_start(vo_v[:, sl], vo)

            # 2-stage software pipeline over the tiles: stage_b of tile t
            # is emitted between stage_a of t+1 and t+2, so every engine's
            # in-order queue interleaves independent work.
            sizes = tile_sizes if tile_sizes is not None else [fd] * nt
            assert sum(sizes) == ntot
            ctxs = []
            off = 0
            nts = len(sizes)
            for i, fdt in enumerate(sizes):
                am = ENG_HEAD if i == 0 else ENG
                ctxs.append(stage_a(i, off, fdt, eng_map=am))
                off += fdt
                if i >= b_lag:
                    stage_b(ctxs[i - b_lag])
            for j in range(nts - b_lag, nts):
                stage_b(ctxs[j],
                        eng_map=ENG_TAIL if j == nts - 1 else ENG)

    _legalize_waits(nc)
    return nc


# ---------------------------------------------------------------- driver
def _make_consts(inputs, states, qMax, Ro, tDiffusion,
                 Wp1, bp1, Wp2, bp2, Wp3, bp3, Wn, bn):
    tD = float(np.asarray(tDiffusion))
    c1 = DT / (VOLB * tD)
    c2 = DT / (VOLS * tD)
    const_q = bool(np.all(qMax == qMax.flat[0]))
    const_ro = bool(np.all(Ro == Ro.flat[0]))
    if const_q:
        qsm = float(qMax.flat[0]) * QMAX_BASE * VOLS / VOL
        rqs = 1.0 / qsm
    else:
        qsm = None
        rqs = None
    kro = float(Ro.flat[0]) * RO_BASE * DT / TO if const_ro else None

    # range of qpS_new for the poly fit (host, cheap)
    i = np.asarray(inputs, np.float64)[:, 0]
    qpB = np.asarray(states, np.float64)[:, 6]
    qpS = np.asarray(states, np.float64)[:, 7]
    qdot = (qpB / VOLB - qpS / VOLS) / tD
    qp_new = qpS + (i + qdot) * DT
    qlo, qhi = float(qp_new.min()), float(qp_new.max())
    margin = max(2.0, 0.25 * (qhi - qlo))
    qlo, qhi = qlo - margin, qhi + margin

    W1 = np.asarray(Wp1, np.float64)
    b1 = np.asarray(bp1, np.float64)
    W2 = np.asarray(Wp2, np.float64)
    b2 = np.asarray(bp2, np.float64)
    W3 = np.asarray(Wp3, np.float64)
    b3 = np.asarray(bp3, np.float64)
    if const_q:
        mid, hw, pc, fit_err = _fit_mlp_poly(qlo, qhi, rqs, W1, b1, W2, b2,
                                             W3, b3)
    else:
        # fit in x-domain over the widest plausible x range
        qsm_all = np.asarray(qMax, np.float64) * 1000.0
        x_new = qp_new / qsm_all
        xlo, xhi = float(x_new.min()), float(x_new.max())
        m = max(0.02, 0.25 * (xhi - xlo))
        xlo, xhi = xlo - m, xhi + m
        mid, hw, pc, fit_err = _fit_mlp_poly(xlo, xhi, 1.0, W1, b1, W2, b2,
                                             W3, b3)

    wn = float(np.asarray(Wn)[0, 0])
    bnv = float(np.asarray(bn)[0])
    wn_eff = wn * rqs if const_q else wn
    k0 = float(pc[0]) + U0P - U0N - bnv

    consts = dict(rqs=rqs, kro=kro, c1=c1, c2=c2,
                  poly=(mid, hw, pc), wn_eff=wn_eff, k0=k0,
                  fit_err=fit_err)
    return consts, (not const_q), (not const_ro)


def kernel(inputs, states, qMax, Ro, tDiffusion,
           Wp1, bp1, Wp2, bp2, Wp3, bp3, Wn, bn,
           _profile=False, _ntot=NTOT, _fd=FD):
    inputs = np.ascontiguousarray(np.asarray(inputs, np.float32))
    states = np.ascontiguousarray(np.asarray(states, np.float32))
    qMax = np.asarray(qMax, np.float32)
    Ro = np.asarray(Ro, np.float32)
    bc = P * _ntot
    assert inputs.shape[0] == NCORES * bc, (inputs.shape, _ntot)

    consts, general_q, general_ro = _make_consts(
        inputs, states, qMax, Ro, tDiffusion,
        Wp1, bp1, Wp2, bp2, Wp3, bp3, Wn, bn)

    sizes = TILE_SIZES if _ntot == NTOT else None
    nc = build_kernel(consts, fd=_fd, ntot=_ntot,
                      general_q=general_q, general_ro=general_ro,
                      it_bufs=2, xo_bufs=3, tile_sizes=sizes)

    in_maps = []
    for c in range(NCORES):
        sl = slice(c * bc, (c + 1) * bc)
        m = {"states": states[sl], "inputs": inputs[sl]}
        if general_q:
            m["qMax"] = np.ascontiguousarray(qMax[sl])
        if general_ro:
            m["Ro"] = np.ascontiguousarray(Ro[sl])
        in_maps.append(m)

    res = run_bass_kernel_spmd(nc, in_maps, core_ids=list(range(NCORES)))
    V = np.concatenate([res.results[c]["V"] for c in range(NCORES)], axis=0)
    Xnew = np.concatenate([res.results[c]["Xnew"] for c in range(NCORES)],
                          axis=0)
    kernel.last_nc = nc
    kernel.last_results = res
    return V, Xnew
